# revision 20
# baseline (speedup 1.0000x reference)
"""Cross-attention kernel for TRN2, batch-parallel over 8 NeuronCores.

Problem shapes (hardcoded): B=8, C1=C2=256, H=W=32 (S=1024), NH=8, KD=VD=64.

Per-core program (core b computes batch element b, no collectives):
  X1 = input1[b] as [C1, S1] (natural layout), X2 likewise.
  K1T = Wk1 @ X1   -> [512, S1]   (head h rows h*64:(h+1)*64)
  K2T = Wk2 @ X2   -> [512, S2]
  V2  = X2.T @ Wv2.T stored per-head with a ones column ([128, 8, 65], bf16)
  heads processed in pairs (2c, 2c+1) sharing K-chunk c, software-pipelined:
    step s2: QK matmuls for both heads (row groups 0/64 run concurrently),
             AV matmuls for step s2-1 (gated on exp), exp(scoresT/8) on ACT.
    A few exp tiles are offloaded to DVE as a cubic Horner polynomial to
    rebalance ACT (the steady-state gate) against the other engines.
    scoresT layout [s2_blk=128, q=1024] avoids all on-chip transposes; the
    plain exp (no max subtraction) equals softmax exactly since scores are
    O(1).  AV lhsT = [v2|1] so PSUM row 64 accumulates the softmax denom.
  normalize: avs=copy(av_psum); rcp=reciprocal_approx_fast(avs);
             denom row -> DRAM -> partition-broadcast back; oall=avs*rcp_rep
             (last pair: PE outer-product broadcast instead of the DRAM hop)
  finalT [C1, S1] = sum_pr WoT_pair.T @ oall_pair  (K=128 head pairs)
  y = finalT reshaped [C1, H, W]  == output[b] layout exactly.

All DRAM inputs are pre-arranged on the host so every input DMA is a
contiguous sweep (no strided descriptors in the prologue).
"""

import sys

for _p in ("/opt/trn_rl_repo", "/root/.axon_site/_ro/trn_rl_repo"):
    if _p not in sys.path:
        sys.path.append(_p)

import numpy as np

import concourse.bass as bass
import concourse.mybir as mybir
import concourse.tile as tile
from concourse import bacc, bass_utils

F32 = mybir.dt.float32
F32R = mybir.dt.float32r
BF16 = mybir.dt.bfloat16
ALU = mybir.AluOpType

B = 8
C1 = 256
S1 = 1024
C2 = 256
S2 = 1024
NH = 8
KD = 64
VD = 64
P = 128

# cubic fit of exp(t/8) on t in [-6, 6] (weighted toward N(0, 0.82) scores)
EC3 = 3.33974236e-04
EC2 = 8.08167160e-03
EC1 = 1.24968101e-01
EC0 = 9.99792011e-01
# (c, s2) pairs whose nh_=1 exp tile runs on DVE instead of ACT
OFFLOAD = set()


def build_nc():
    nc = bacc.Bacc(
        "TRN2",
        target_bir_lowering=False,
        debug=False,
        enable_asserts=False,
        num_devices=B,
    )

    x1 = nc.dram_tensor("x1", [C1, S1], BF16, kind="ExternalInput").ap()
    x2 = nc.dram_tensor("x2", [C2, S2], BF16, kind="ExternalInput").ap()
    wkv = nc.dram_tensor("wkv", [3, C1, NH * KD], BF16, kind="ExternalInput").ap()
    wot = nc.dram_tensor("wot", [NH * VD, C1], F32R, kind="ExternalInput").ap()
    y = nc.dram_tensor("y", [C1, S1], F32, kind="ExternalOutput").ap()

    with tile.TileContext(nc) as tc:
        with (
            tc.tile_pool(name="const", bufs=1) as cpool,
            tc.tile_pool(name="expt", bufs=7) as epool,
            tc.tile_pool(name="xt", bufs=3) as xpool,
            tc.tile_pool(name="norm", bufs=2) as npool,
            tc.tile_pool(name="yout", bufs=2) as ypool,
            tc.tile_pool(name="pmm", bufs=2, space="PSUM") as pmm,
            tc.tile_pool(name="pav", bufs=2, space="PSUM") as pav,
            tc.tile_pool(name="dscr", bufs=2, space="DRAM") as dpool,
        ):
            # ---- load inputs (all contiguous DMAs) ----
            x1_big = cpool.tile([P, 2, S1], BF16, name="x1_big")
            x2_big = cpool.tile([P, 2, S2], BF16, name="x2_big")
            wkv_sb = cpool.tile([P, 3, 2, 512], BF16, name="wkv_sb")
            wot_sb = cpool.tile([64, NH, C1], F32R, name="wot_sb")
            nc.gpsimd.dma_start(
                wkv_sb[:], wkv.rearrange("t (c p) f -> p t c f", p=P)
            )
            nc.sync.dma_start(x1_big[:], x1.rearrange("(c p) s -> p c s", p=P))
            nc.sync.dma_start(x2_big[:], x2.rearrange("(c p) s -> p c s", p=P))
            nc.gpsimd.dma_start(
                wot_sb[:], wot.rearrange("(h r) c -> r h c", r=64)
            )
            x1_sb = [x1_big[:, c, :] for c in range(2)]
            x2_sb = [x2_big[:, c, :] for c in range(2)]
            wk1t_sb = [wkv_sb[:, 0, c, :] for c in range(2)]
            wk2t_sb = [wkv_sb[:, 1, c, :] for c in range(2)]
            wv2t_sb = [wkv_sb[:, 2, c, :] for c in range(2)]

            k1t_sb = [cpool.tile([P, S1], BF16, name=f"k1t_{m}") for m in range(4)]
            k2t_sb = [cpool.tile([P, S2], BF16, name=f"k2t_{m}") for m in range(4)]
            # v2 with per-head ones column: [128, head, 65]
            v2a_sb = [
                cpool.tile([P, NH, VD + 1], BF16, name=f"v2a_{s}") for s in range(8)
            ]
            oall_sb = [cpool.tile([64, S1], F32R, name=f"oall_{h}") for h in range(NH)]

            def emit_proj_chunk(pool, wt_sb, xs_sb, kt, m, eng="vec"):
                """kt[m] (bf16 [128, S]) = (wt chunk).T @ xs."""
                tag = "qk" if pool is pmm else "pav"
                ps = pool.tile([P, 1024], F32, tag=tag, name=f"pj_{kt[m].name}")
                for nh_ in range(2):
                    for k in range(2):
                        nc.tensor.matmul(
                            ps[:, nh_ * 512 : (nh_ + 1) * 512],
                            wt_sb[k][:, m * P : (m + 1) * P],
                            xs_sb[k][:, nh_ * 512 : (nh_ + 1) * 512],
                            start=(k == 0),
                            stop=(k == 1),
                        )
                if eng == "act":
                    nc.scalar.copy(out=kt[m][:], in_=ps[:])
                else:
                    nc.vector.tensor_copy(out=kt[m][:], in_=ps[:])

            def emit_v2_pair(sp):
                ps = pav.tile([P, 1024], F32, tag="pav", name=f"pv2_{sp}")
                for half in range(2):
                    s = 2 * sp + half
                    for k in range(2):
                        nc.tensor.matmul(
                            ps[:, half * 512 : (half + 1) * 512],
                            x2_sb[k][:, s * P : (s + 1) * P],
                            wv2t_sb[k][:],
                            start=(k == 0),
                            stop=(k == 1),
                        )
                for half in range(2):
                    s = 2 * sp + half
                    nc.vector.memset(v2a_sb[s][:, :, VD : VD + 1], 1.0)
                    nc.vector.tensor_copy(
                        out=v2a_sb[s][:, :, 0:VD],
                        in_=ps[:, half * 512 : (half + 1) * 512].rearrange(
                            "p (h c) -> p h c", c=VD
                        ),
                    )

            # ---- prologue: K-chunk 0 projections (attention gates on these) ----
            # k1t on ACT concurrently with k2t on DVE
            emit_proj_chunk(pmm, wk1t_sb, x1_sb, k1t_sb, 0, eng="vec")
            emit_proj_chunk(pmm, wk2t_sb, x2_sb, k2t_sb, 0, eng="vec")

            # ---- attention: pair-packed (a|b) flat pipeline ----
            av_tiles = {}
            et_tiles = {}
            pending = []

            def emit_av(c, s2):
                a, b = 2 * c, 2 * c + 1
                if s2 == 0:
                    for h in (a, b):
                        av_tiles[h] = pav.tile(
                            [VD + 1, S1], F32, tag="pav", name=f"av_{h}"
                        )
                for nh_ in range(2):
                    et = et_tiles[(c, s2, nh_)]
                    for idx, h in enumerate((a, b)):
                        nc.tensor.matmul(
                            av_tiles[h][:, nh_ * 512 : (nh_ + 1) * 512],
                            v2a_sb[s2][:, h, :],
                            et[:, idx * 512 : (idx + 1) * 512],
                            start=(s2 == 0),
                            stop=(s2 == 7),
                            skip_group_check=True,
                        )
                for nh_ in range(2):
                    del et_tiles[(c, s2, nh_)]

            def oall_write(h, eng, avs, rep):
                if eng == "gps":
                    nc.gpsimd.tensor_mul(
                        out=oall_sb[h][:], in0=avs[0:VD, :], in1=rep[:]
                    )
                else:
                    nc.vector.tensor_mul(
                        out=oall_sb[h][:], in0=avs[0:VD, :], in1=rep[:]
                    )

            def emit_normalize(h):
                """DMA-broadcast path for mid-loop heads (latency hidden)."""
                avs = npool.tile([VD + 1, S1], F32, tag="avs", name=f"avs_{h}")
                nc.vector.tensor_copy(out=avs[:], in_=av_tiles[h][:])
                rcp = npool.tile([VD + 1, S1], F32, tag="rcp", name=f"rcp_{h}")
                nc.vector.reciprocal_approx_fast(rcp[:], avs[:])
                rdram = dpool.tile([S1], F32, tag="rd", name=f"rd_{h}")
                nc.sync.dma_start(rdram[:], rcp[VD : VD + 1, :])
                rep = npool.tile([64, S1], F32, tag="rep", name=f"rep_{h}")
                nc.sync.dma_start(rep[:], rdram[None, :].to_broadcast((64, S1)))
                oall_write(h, "gps", avs, rep)

            def emit_normalize_fast(h, copy_eng):
                """Tail path: parallel ACT/DVE copies, muls on DVE."""
                avs = npool.tile([VD + 1, S1], F32, tag="avs", name=f"avs_{h}")
                if copy_eng == "act":
                    nc.scalar.copy(out=avs[:], in_=av_tiles[h][:])
                else:
                    nc.vector.tensor_copy(out=avs[:], in_=av_tiles[h][:])
                rcp = npool.tile([VD + 1, S1], F32, tag="rcp", name=f"rcp_{h}")
                nc.vector.reciprocal_approx_fast(
                    rcp[VD : VD + 1, :], avs[VD : VD + 1, :]
                )
                rdram = dpool.tile([S1], F32, tag="rd", name=f"rd_{h}")
                nc.sync.dma_start(rdram[:], rcp[VD : VD + 1, :])
                rep = npool.tile([64, S1], F32, tag="rep", name=f"rep_{h}")
                nc.sync.dma_start(rep[:], rdram[None, :].to_broadcast((64, S1)))
                oall_write(h, "vec", avs, rep)

            def flush_av(upto):
                while len(pending) > upto:
                    cc, ss = pending.pop(0)
                    emit_av(cc, ss)
                    if ss == 7:
                        if cc == 3:
                            emit_normalize_fast(6, "vec")
                            emit_normalize_fast(7, "act")
                        else:
                            emit_normalize(2 * cc)
                            emit_normalize(2 * cc + 1)
                        if cc + 2 <= 3:
                            emit_proj_chunk(pav, wk1t_sb, x1_sb, k1t_sb, cc + 2)
                            emit_proj_chunk(pav, wk2t_sb, x2_sb, k2t_sb, cc + 2)

            def emit_exp_dve(qk, et):
                """et = cubic(qk) ~= exp(qk/8) on DVE, Horner via fused stt ops."""
                xb = xpool.tile([P, S1], BF16, tag="xb", name="exp_x")
                u1 = xpool.tile([P, S1], BF16, tag="xb", name="exp_u1")
                u2 = xpool.tile([P, S1], BF16, tag="xb", name="exp_u2")
                nc.vector.tensor_copy(out=xb[:], in_=qk[:])
                nc.vector.scalar_tensor_tensor(
                    u1[:], xb[:], EC2 / EC3, xb[:], ALU.add, ALU.mult
                )
                nc.vector.scalar_tensor_tensor(
                    u2[:], u1[:], EC1 / EC3, xb[:], ALU.add, ALU.mult
                )
                nc.vector.tensor_scalar(et[:], u2[:], EC3, EC0, ALU.mult, ALU.add)

            for c in range(4):
                a, b = 2 * c, 2 * c + 1
                for s2 in range(8):
                    qks = []
                    for nh_ in range(2):
                        qk = pmm.tile(
                            [P, S1], F32, tag="qk", name=f"qk_{c}_{s2}_{nh_}"
                        )
                        for idx, h in enumerate((a, b)):
                            ro = (h % 2) * 64
                            nc.tensor.matmul(
                                qk[:, idx * 512 : (idx + 1) * 512],
                                k2t_sb[c][ro : ro + 64, s2 * P : (s2 + 1) * P],
                                k1t_sb[c][ro : ro + 64, nh_ * 512 : (nh_ + 1) * 512],
                                start=True,
                                stop=True,
                            )
                        qks.append(qk)
                    if c == 0:
                        if s2 == 0:
                            emit_v2_pair(0)
                            emit_v2_pair(1)
                        elif s2 == 1:
                            emit_v2_pair(2)
                            emit_v2_pair(3)
                        elif s2 == 2:
                            emit_proj_chunk(pav, wk1t_sb, x1_sb, k1t_sb, 1)
                            emit_proj_chunk(pav, wk2t_sb, x2_sb, k2t_sb, 1)
                    flush_av(2 if c == 0 else 1)
                    for nh_ in range(2):
                        et = epool.tile(
                            [P, S1], BF16, tag="expt", name=f"et_{c}_{s2}_{nh_}"
                        )
                        if nh_ == 1 and (c, s2) in OFFLOAD:
                            emit_exp_dve(qks[nh_], et)
                        else:
                            nc.scalar.activation(
                                et[:],
                                qks[nh_][:],
                                mybir.ActivationFunctionType.Exp,
                                scale=0.125,
                            )
                        et_tiles[(c, s2, nh_)] = et
                    pending.append((c, s2))
            flush_av(0)

            # ---- final projection: y[mt] = sum_pr WoT_pair.T @ oall_pair ----
            fins = [
                pmm.tile([P, S1], F32, tag="qk", name=f"fin_{mt}") for mt in range(2)
            ]

            def fin_mms(mt, hs):
                for h in hs:
                    for nh_ in range(2):
                        nc.tensor.matmul(
                            fins[mt][:, nh_ * 512 : (nh_ + 1) * 512],
                            wot_sb[:, h, mt * P : (mt + 1) * P],
                            oall_sb[h][:, nh_ * 512 : (nh_ + 1) * 512],
                            start=(h == 0),
                            stop=(h == NH - 1),
                            skip_group_check=True,
                        )

            fin_mms(0, range(6))
            fin_mms(1, range(6))

            def ship_y(mt):
                ysb = ypool.tile([P, S1], F32, tag=f"y{mt}", name=f"y_{mt}")
                nc.scalar.copy(out=ysb[:], in_=fins[mt][:])
                nc.sync.dma_start(y[mt * P : (mt + 1) * P, :], ysb[:])

            fin_mms(0, (6, 7))
            ship_y(0)
            fin_mms(1, (6, 7))
            ship_y(1)

    nc.compile()
    return nc


_nc_cache = None


def _get_nc():
    global _nc_cache
    if _nc_cache is None:
        _nc_cache = build_nc()
    return _nc_cache


def _make_in_maps(input1, input2, Wk1, Wk2, Wv2, Wo):
    import ml_dtypes

    bf16 = ml_dtypes.bfloat16
    x1 = np.asarray(input1, dtype=np.float32).astype(bf16)
    x2 = np.asarray(input2, dtype=np.float32).astype(bf16)
    wkv = np.ascontiguousarray(
        np.stack(
            [np.asarray(W, dtype=np.float32).T.astype(bf16) for W in (Wk1, Wk2, Wv2)]
        )
    )
    wot = np.ascontiguousarray(np.asarray(Wo, dtype=np.float32).T)
    return [
        {
            "x1": np.ascontiguousarray(x1[b].reshape(C1, S1)),
            "x2": np.ascontiguousarray(x2[b].reshape(C2, S2)),
            "wkv": wkv,
            "wot": wot,
        }
        for b in range(B)
    ]


def _assemble(results):
    out = np.stack([results[b]["y"] for b in range(B)], axis=0)
    return np.ascontiguousarray(out.reshape(B, C1, 32, 32).astype(np.float32))


def kernel(input1, input2, Wk1, Wk2, Wv2, Wo):
    nc = _get_nc()
    in_maps = _make_in_maps(input1, input2, Wk1, Wk2, Wv2, Wo)
    res = bass_utils.run_bass_kernel_spmd(nc, in_maps, core_ids=list(range(B)))
    return _assemble(res.results)


def kernel_traced(input1, input2, Wk1, Wk2, Wv2, Wo):
    """Like kernel() but with NTFF profiling; returns (out, BassKernelResults)."""
    nc = _get_nc()
    in_maps = _make_in_maps(input1, input2, Wk1, Wk2, Wv2, Wo)
    res = bass_utils.run_bass_kernel_spmd(
        nc, in_maps, core_ids=list(range(B)), trace=True
    )
    return _assemble(res.results), res
